# revision 12
# baseline (speedup 1.0000x reference)
"""Trainium2 Bass kernel for the CIDER GNN-message-passing head, v7.

Math (identical reduction to the reference):
  neigh[b]  = sum_{h<96} hist[b,h,:]
  c[b]      = neigh[b] @ (W_l.T/96) + b_l
  Q[b]      = hist[b,0,:] @ (W_r.T W_Q.T) + neigh[b] @ (W_l.T W_Q.T)/96
              + (b_l @ W_Q.T + b_Q)
  u[b]      = Q[b] @ (W_K @ W_r)
  sc[b,h]   = hist[b,h,:] . u[b]
  a[b,h]    = exp(sc/16) / Z[b]
  out[b]    = (sum_h a[b,h] hist[b,h,:]) @ W_r.T + c[b]
  output    = broadcast out over the candidate dim.

v7 strategy (vs v6's dual-precision hist):
  - hist ships ONCE logically: error-feedback (EF) e3m4 quantization along
    h (carry per (b,d) lane).  Because alpha ~= 1/H (scores are small) and
    EF makes prefix sums over h near-exact, the SAME 1-byte values serve
    all three hist uses: scores (fp8-tolerant), neigh (EF sum ~exact), and
    hbar (mean-dominated + EF kills the mean error).  Two layouts of those
    bytes ship (h-major for PE sums, d-major for scores) = 1.84 MB vs
    v6's 2.76 MB.
  - The DVE pairwise tree dies: one PE ones-matmul neigh (f32 psum) feeds
    both the Q path and the c path.
  - W_r.T ships as a single e3m4 tensor (no residual); W_l.T stays f16.
    Simulated end-to-end rel err ~1.1e-2 vs the 2e-2 gate (v6: 4e-3).
  - Junk warmer matmuls keyed to DMA arrivals hold the PE p-state era
    open across idle windows.

Sharding: pure data parallel, batch 96 -> 12 samples on each of 8 cores;
weights replicated; no collectives.
"""

import os
import sys

for _p in ("/opt/trn_rl_repo", "/root/.axon_site/_ro/trn_rl_repo"):
    if os.path.isdir(_p) and _p not in sys.path:
        sys.path.insert(0, _p)

import ml_dtypes
import numpy as np

import concourse.bacc as bacc
import concourse.tile as tile
from concourse import mybir
from concourse.bass_utils import run_bass_kernel_spmd

B, H, NCAND, D, A = 96, 100, 128, 768, 256
NCORES = 8
BC = B // NCORES          # 12 samples per core
DC = D // 128             # 6 chunks of the 768-dim
AC = A // 128             # 2 chunks of the 256-dim
NG = 3                    # d-major hist sample groups of 4
JW = 3                    # weight j-slice groups (2 output chunks each)
GS = BC // NG

F32 = mybir.dt.float32
F16 = mybir.dt.float16
DT = F16
X = mybir.AxisListType.X
ALU = mybir.AluOpType
ACTF = mybir.ActivationFunctionType

# --- precision knobs ---------------------------------------------------
HIST_E3 = False            # EF hist dtype: e3m4 (else e4m3)
WRT_E3 = True             # W_r.T: single e3m4 (else e4m3 base + residual)
SH = np.float32(2.0) if HIST_E3 else np.float32(16.0)   # hist scale
S1 = np.float32(64.0)     # fw / wrt-residual scale
S2 = np.float32(4096.0)   # tiny W_l.T W_Q.T/96 scale
SW3 = np.float32(96.0)    # wrt e3m4 scale (max|W_r| ~0.097 -> ~9.3 < 15.5)

F8H = mybir.dt.float8e3 if HIST_E3 else mybir.dt.float8e4
F8W = mybir.dt.float8e3 if WRT_E3 else mybir.dt.float8e4
F8 = mybir.dt.float8e4
NPF8 = ml_dtypes.float8_e4m3
NPF8H = ml_dtypes.float8_e3m4 if HIST_E3 else ml_dtypes.float8_e4m3
NPF8W = ml_dtypes.float8_e3m4 if WRT_E3 else ml_dtypes.float8_e4m3

_CACHE = {}


def _build():
    nc = bacc.Bacc(
        "TRN2",
        target_bir_lowering=False,
        debug=False,
        enable_asserts=True,
        num_devices=NCORES,
    )
    sm16_d = nc.dram_tensor("sm16", [128, 256], DT, kind="ExternalInput")
    fw_d = nc.dram_tensor("fw", [128, 3 * DC * A], F8, kind="ExternalInput")
    hh_d = nc.dram_tensor("hh", [H, BC * D], F8H, kind="ExternalInput")
    hd_d = nc.dram_tensor("hd", [128, NG * DC * GS * H], F8H, kind="ExternalInput")
    wlt_d = nc.dram_tensor("wlt", [128, DC * D], DT, kind="ExternalInput")
    wrt8_d = nc.dram_tensor("wrt8", [128, DC * D], F8W, kind="ExternalInput")
    if not WRT_E3:
        wrtd_d = nc.dram_tensor("wrtd", [128, DC * D], F8, kind="ExternalInput")
    out_d = nc.dram_tensor("out", [128, DC * BC], DT, kind="ExternalOutput")

    with tile.TileContext(nc) as tc:
        with (
            tc.tile_pool(name="sp", bufs=1) as sp,
            tc.tile_pool(name="pW", bufs=1, space="PSUM") as pW,
            tc.tile_pool(name="pN", bufs=1, space="PSUM") as pN,
            tc.tile_pool(name="pA", bufs=2, space="PSUM") as pA,
            tc.tile_pool(name="pS", bufs=2, space="PSUM") as pS,
            tc.tile_pool(name="pH", bufs=1, space="PSUM") as pH,
            tc.tile_pool(name="pO", bufs=1, space="PSUM") as pO,
        ):
            # ------- DMAs in chain-consumption order ------------------------
            # h-major EF hist first (big transfer covers later DMAs' HWDGE
            # pipeline); feeds neigh + hbar
            hh = sp.tile([H, BC, DC, 128], F8H, name="hh", tag="hh")
            nc.sync.dma_start(
                hh[:], hh_d[:].rearrange("p (b c e) -> p b c e", b=BC, c=DC)
            )
            sm16 = sp.tile([128, 256], DT, name="sm16", tag="sm16")
            nc.sync.dma_start(sm16[:], sm16_d[:])
            fw = sp.tile([128, 3 * DC * A], F8, name="fw", tag="fw")
            nc.sync.dma_start(fw[:], fw_d[:])
            f1 = fw[:, 0:DC * A].rearrange("p (c a) -> p c a", c=DC)
            f2 = fw[:, DC * A:2 * DC * A].rearrange("p (c a) -> p c a", c=DC)
            wg = fw[:, 2 * DC * A:].rearrange("p (c e) -> p c e", c=AC)
            # d-major EF hist in 3 sample groups, feeds scores
            hd = sp.tile([128, NG, DC, GS, H], F8H, name="hd", tag="hd")
            hdv = hd_d[:].rearrange("p (g c s h) -> p g c s h", g=NG, c=DC, s=GS)
            nc.sync.dma_start(hd[:, 0], hdv[:, 0])
            nc.sync.dma_start(hd[:, 1], hdv[:, 1])
            nc.sync.dma_start(hd[:, 2], hdv[:, 2])
            # weights j-sliced (output-column pairs) so out chunks compute as
            # the stream lands; each 2j-slice is self-contained
            wlt = sp.tile([128, JW, DC, 256], DT, name="wlt", tag="wlt")
            wrt8 = sp.tile([128, JW, DC, 256], F8W, name="wrt8", tag="wrt8")
            wltv = wlt_d[:].rearrange("p (w c e) -> p w c e", w=JW, c=DC)
            wrtv = wrt8_d[:].rearrange("p (w c e) -> p w c e", w=JW, c=DC)
            for w in range(JW):
                nc.sync.dma_start(wlt[:, w], wltv[:, w])
                nc.sync.dma_start(wrt8[:, w], wrtv[:, w])

            h0T = sm16[:, 0:DC * BC].rearrange("p (c b) -> p c b", c=DC)
            ident = sm16[0:BC, 72:72 + BC]
            ident100 = sm16[0:H, 72:72 + H]
            blT = sm16[:, 200:200 + DC]
            bqlT = sm16[:, 206:206 + AC]
            ones96 = sm16[0:H, 208:209]
            ones100 = sm16[0:H, 209:210]

            # ------- PE p-state warmers: junk matmuls keyed to arrivals ------
            junk = sp.tile([1, 128], DT, name="junk", tag="junk")
            nc.vector.memset(junk[:], 1.0)
            ps_w0 = pW.tile([128, 128], F32, name="wm0", tag="wm")
            nc.tensor.matmul(ps_w0[:], junk[:], junk[:], start=True, stop=True)
            junk2 = sp.tile([1, 128], DT, name="junk2", tag="junk2")
            nc.vector.tensor_copy(junk2[:], ps_w0[0:1, :])
            warm_rhs = [
                junk2[:],                  # chained era keep-alive
                hh[0:1, 0, 0, :],
                sm16[0:1, 0:128],
                fw[0:1, 3072:3200],
                hd[0:1, 0, 0, 0, :],
            ]
            for i, rhs in enumerate(warm_rhs):
                ps_w = pW.tile([128, 128], F32, name=f"wm{i + 1}", tag="wm")
                nc.tensor.matmul(
                    ps_w[:, 0:rhs.shape[-1]], junk[:], rhs, start=True, stop=True
                )

            JB = [slice(128 * j, 128 * (j + 1)) for j in range(DC)]

            # ---- neigh (EF sum over h<96) from h-major hist: PE ones-matmul.
            # Stored values are SH*x, so psum = SH*neigh.
            neighF_ps = pN.tile([128, DC, BC], F32, name="nF_ps", tag="pn")
            for k in range(DC):
                for b in range(BC):
                    nc.tensor.matmul(
                        neighF_ps[:, k, b:b + 1], hh[:, b, k, :], ones96,
                        start=True, stop=True,
                    )
            neighF = sp.tile([128, DC, BC], DT, name="nF", tag="nF")
            nc.vector.tensor_copy(neighF[:], neighF_ps[:])

            # ---- QT = (S1 W_r.T W_Q.T) @ h0T / S1
            # ----    + (S2 W_l.T W_Q.T/96) @ (SH neigh) / (S2 SH) + bql ------
            A_ps = pA.tile([128, AC, BC], F32, name="A_ps", tag="pa")
            B_ps = pA.tile([128, AC, BC], F32, name="B_ps", tag="pa")
            for j in range(AC):
                for k in range(DC):
                    nc.tensor.matmul(
                        A_ps[:, j, :], f1[:, k, 128 * j:128 * (j + 1)],
                        h0T[:, k, :],
                        start=(k == 0), stop=(k == DC - 1),
                    )
            for j in range(AC):
                for k in range(DC):
                    nc.tensor.matmul(
                        B_ps[:, j, :], f2[:, k, 128 * j:128 * (j + 1)],
                        neighF[:, k, :],
                        start=(k == 0), stop=(k == DC - 1),
                    )
            Qt1 = sp.tile([128, AC, BC], F32, name="Qt1", tag="Qt1")
            nc.vector.scalar_tensor_tensor(
                Qt1[:], B_ps[:], float(1.0 / (S2 * SH)),
                bqlT.unsqueeze(2).broadcast_to([128, AC, BC]),
                op0=ALU.mult, op1=ALU.add,
            )
            QT = sp.tile([128, AC, BC], DT, name="QT", tag="QT")
            nc.vector.scalar_tensor_tensor(
                QT[:], A_ps[:], float(1.0 / S1), Qt1[:],
                op0=ALU.mult, op1=ALU.add,
            )

            # ---- uT = (S1 W_K W_r) @ QT / S1  [128(e), 6, 12] ---------------
            uT_ps = pA.tile([128, DC, BC], F32, name="uT_ps", tag="pa")
            for j in range(DC):
                for k in range(AC):
                    nc.tensor.matmul(
                        uT_ps[:, j, :], wg[:, k, JB[j]], QT[:, k, :],
                        start=(k == 0), stop=(k == AC - 1),
                    )
            uT = sp.tile([128, DC, BC], DT, name="uT", tag="uT")
            nc.vector.tensor_scalar_mul(uT[:], uT_ps[:], float(1.0 / S1))

            # ---- scores transposed: 72 single-column matmuls over d ---------
            # psum = SH * sc
            scT_ps = pS.tile([H, BC], F32, name="scT_ps", tag="sc")
            for b in range(BC):
                for k in range(DC):
                    nc.tensor.matmul(
                        scT_ps[:, b:b + 1], hd[:, b // GS, k, b % GS, :],
                        uT[:, k, b:b + 1],
                        start=(k == 0), stop=(k == DC - 1),
                    )

            # ---- softmax, short chain: per-group exp + Z; hbar stays
            # ---- UNNORMALIZED (weights = esc); 1/Z applied after via a
            # ---- PE-broadcast reciprocal.  Saves two PE transposes and
            # ---- several serial sem hops.
            escT = sp.tile([H, BC], DT, name="escT", tag="escT")
            Z_ps = pS.tile([BC, 1], F32, name="Z_ps", tag="sc")
            hbar_ps = pH.tile([128, DC, BC], F32, name="hb_ps", tag="ph")
            for g in range(NG):
                gs = slice(GS * g, GS * (g + 1))
                nc.scalar.activation(
                    escT[:, gs], scT_ps[:, gs], ACTF.Exp,
                    scale=float(1.0 / (16.0 * SH)),
                )
                for k in range(DC):
                    for b in range(GS * g, GS * (g + 1)):
                        nc.tensor.matmul(
                            hbar_ps[:, k, b:b + 1], hh[:, b, k, :],
                            escT[:, b:b + 1], start=True, stop=True,
                        )
            nc.tensor.matmul(Z_ps[:], escT[:], ones100, start=True, stop=True)
            recip = sp.tile([BC, 1], DT, name="recip", tag="recip")
            with nc.allow_low_precision(reason="1/Z fits f16; |Z| in [30, 300]"):
                nc.vector.reciprocal(recip[:], Z_ps[:])
            rT_ps = pS.tile([1, BC], DT, name="rT_ps", tag="sc")
            nc.tensor.transpose(rT_ps[:], recip[:], ident)
            rT = sp.tile([1, BC], DT, name="rT", tag="rT")
            nc.vector.tensor_copy(rT[:], rT_ps[:])
            rbc_ps = pS.tile([128, BC], F32, name="rbc_ps", tag="sc")
            nc.tensor.matmul(rbc_ps[:], junk[:], rT[:], start=True, stop=True)
            rbc = sp.tile([128, BC], DT, name="rbc", tag="rbcs")
            nc.vector.tensor_scalar_mul(rbc[:], rbc_ps[:], float(1.0 / (SW3 * SH)))
            hbarT = sp.tile([128, DC, BC], DT, name="hbT", tag="hbT")
            nc.vector.tensor_mul(
                hbarT[:], hbar_ps[:],
                rbc[:].unsqueeze(1).broadcast_to([128, DC, BC]),
            )

            # ---- per-2j-slice as weights land: ONE psum accumulates both
            # ---- c_j = (W_l.T/(96 SH)) @ (SH neigh) and the true-scale
            # ---- (SW3 W_r.T) @ (hbar/(SW3 Z)) attention part; epilogue is a
            # ---- single bias-add per slice.
            o_ps = pO.tile([128, DC, BC], F32, name="o_ps", tag="po")
            out_sb = sp.tile([128, DC, BC], DT, name="out_sb", tag="out_sb")
            # output write via SWDGE prepare+trigger: descriptors are
            # generated early; the trigger fires as soon as out_sb is ready,
            # skipping the HWDGE dispatch+DGE latency on the critical tail
            USE_KVWB = False
            if USE_KVWB:
                ctx0 = sp.tile([128, 1], mybir.dt.int32, name="ctx0", tag="ctx0")
                nc.vector.memset(ctx0[:], 0)
                out4 = out_d[:].rearrange("(a p) (b f) -> a p b f", a=1, b=1)
                in4 = out_sb[:].rearrange("p c b -> p (c b)").rearrange(
                    "p (x y f) -> p x y f", x=1, y=1
                )
                out_dma_sem = nc.alloc_semaphore("out_dma_sem")
                nc.gpsimd.kv_writeback(
                    out4, in4, ctx0[:], prepare_only=True, sem=out_dma_sem
                )
            for w in range(JW):
                for jj in range(2):
                    j = 2 * w + jj
                    jsl = slice(128 * jj, 128 * (jj + 1))
                    for k in range(DC):
                        nc.tensor.matmul(
                            o_ps[:, j, :], wlt[:, w, k, jsl], neighF[:, k, :],
                            start=(k == 0), stop=False,
                        )
                    for k in range(DC):
                        nc.tensor.matmul(
                            o_ps[:, j, :], wrt8[:, w, k, jsl], hbarT[:, k, :],
                            start=False, stop=(k == DC - 1),
                        )
                jw = slice(2 * w, 2 * w + 2)
                nc.vector.tensor_add(
                    out_sb[:, jw, :], o_ps[:, jw, :],
                    blT[:, jw].unsqueeze(2).broadcast_to([128, 2, BC]),
                )
            if USE_KVWB:
                nc.gpsimd.trigger_dma(count=None)
                nc.gpsimd.wait_ge(out_dma_sem, 16)
            else:
                nc.sync.dma_start(out_d[:], out_sb[:].rearrange("p c b -> p (c b)"))

    nc.compile()
    return nc


def _get_nc():
    if "nc" not in _CACHE:
        _CACHE["nc"] = _build()
    return _CACHE["nc"]


def _chunked_T(m, dtype=np.float16, scale=None):
    """[R*128, C] -> [128, R, C] -> [128, R*C] (d-on-partitions layout)."""
    r = m.shape[0] // 128
    if scale is not None:
        m = m * scale
    return np.ascontiguousarray(
        m.reshape(r, 128, -1).transpose(1, 0, 2).reshape(128, -1).astype(dtype)
    )


def _ef_quant(hist, dt, scale):
    """Error-feedback quantize along h (axis=1): per-(b,d) carry keeps all
    prefix sums over h near-exact. Returns the STORED (scaled) fp8 array."""
    Bq, Hq, Dq = hist.shape
    out = np.empty((Bq, Hq, Dq), dt)
    carry = np.zeros((Bq, Dq), np.float32)
    hs = hist * np.float32(scale)
    for h in range(Hq):
        tgt = hs[:, h, :] + carry
        qv = np.asarray(tgt, dtype=dt)
        carry = tgt - qv.astype(np.float32)
        out[:, h, :] = qv
    return out


def _prep_in_maps(inputs):
    hist = np.asarray(inputs["history_embedding"], dtype=np.float32)
    W_l = np.asarray(inputs["W_l"], dtype=np.float32)
    b_l = np.asarray(inputs["b_l"], dtype=np.float32)
    W_r = np.asarray(inputs["W_r"], dtype=np.float32)
    W_K = np.asarray(inputs["W_K"], dtype=np.float32)
    W_Q = np.asarray(inputs["W_Q"], dtype=np.float32)
    b_Q = np.asarray(inputs["b_Q"], dtype=np.float32)

    bql = b_l @ W_Q.T + b_Q

    def _jsliced(m, dtype, scale):
        """[768, 768] -> [128, JW, DC, 256]: w-th slice holds output columns
        [256w, 256w+256) for all six 128-row contraction chunks."""
        t = (m * scale).reshape(DC, 128, JW, 256)   # [k, p, w, e]
        return np.ascontiguousarray(
            t.transpose(1, 2, 0, 3).reshape(128, -1).astype(dtype)
        )

    shared = {
        "wlt": _jsliced(W_l.T / np.float32(96.0 * SH), np.float16, 1.0),
        "wrt8": _jsliced(W_r.T, NPF8W, SW3),
        "fw": np.concatenate([
            _chunked_T(W_r.T @ W_Q.T, dtype=NPF8, scale=S1),
            _chunked_T(W_l.T @ W_Q.T / np.float32(96.0), dtype=NPF8, scale=S2),
            _chunked_T(W_K @ W_r, dtype=NPF8, scale=S1),
        ], axis=1),
    }

    hq = _ef_quant(hist, NPF8H, SH)      # [96, 100, 768] stored fp8
    in_maps = []
    for i in range(NCORES):
        m = dict(shared)
        hs = hq[i * BC:(i + 1) * BC]     # [12, 100, 768] fp8
        # h-major: [H, BC*768] (b, then d-chunk, then e contiguous)
        m["hh"] = np.ascontiguousarray(
            hs.transpose(1, 0, 2).reshape(H, BC * D)
        )
        # d-major: [128, NG*DC*GS*H]
        m["hd"] = np.ascontiguousarray(
            hs.transpose(2, 0, 1)                 # [768, 12, 100]
            .reshape(DC, 128, NG, GS, H)
            .transpose(1, 2, 0, 3, 4)             # [128, g, k, s, h]
            .reshape(128, -1)
        )
        sm16 = np.zeros((128, 256), np.float16)
        sm16[:, 0:DC * BC] = (
            hist[i * BC:(i + 1) * BC, 0, :]
            .T.reshape(DC, 128, BC).transpose(1, 0, 2).reshape(128, -1)
        )
        sm16[:, 72:200] = np.eye(128, dtype=np.float16)
        sm16[:, 200:200 + DC] = b_l.reshape(DC, 128).T
        sm16[:, 206:206 + AC] = bql.reshape(AC, 128).T
        sm16[0:96, 208] = 1.0
        sm16[0:H, 209] = 1.0
        m["sm16"] = sm16
        in_maps.append(m)
    return in_maps


def run_device(inputs, trace=False, **kwargs):
    """Returns (out [96,768] float32, BassKernelResults)."""
    nc = _get_nc()
    in_maps = _prep_in_maps(inputs)
    try:
        res = run_bass_kernel_spmd(
            nc, in_maps, core_ids=list(range(NCORES)), trace=trace, **kwargs
        )
    except Exception:
        # transient NRT_EXEC_UNIT_UNRECOVERABLE from a wedged device has been
        # observed on first-touch; one retry reliably recovers
        res = run_bass_kernel_spmd(
            nc, in_maps, core_ids=list(range(NCORES)), trace=trace, **kwargs
        )
    outs = []
    for i in range(NCORES):
        o = np.asarray(res.results[i]["out"], dtype=np.float32)
        outs.append(o.reshape(128, DC, BC).transpose(2, 1, 0).reshape(BC, D))
    return np.concatenate(outs, axis=0), res


def kernel(**inputs):
    out, _ = run_device(inputs)
    full = np.broadcast_to(out[:, None, :], (B, NCAND, D))
    return np.ascontiguousarray(full)


if __name__ == "__main__":
    rng = np.random.default_rng(0)
    ins = {
        "history_embedding": rng.standard_normal((B, H, D)).astype(np.float32),
        "candidate_news_representation": rng.standard_normal((B, NCAND, D)).astype(np.float32),
        "W_l": (rng.standard_normal((D, D)) * 0.02).astype(np.float32),
        "b_l": np.zeros(D, np.float32),
        "W_r": (rng.standard_normal((D, D)) * 0.02).astype(np.float32),
        "W_K": (rng.standard_normal((A, D)) * 0.02).astype(np.float32),
        "W_Q": (rng.standard_normal((A, D)) * 0.02).astype(np.float32),
        "b_Q": np.zeros(A, np.float32),
    }
    out = kernel(**ins)
    print("kernel ran, output", out.shape, out.dtype)
